# revision 17
# baseline (speedup 1.0000x reference)
"""MoE ConditionalFeedForward (int8 SwiGLU experts) on 8 trn2 NeuronCores.

Expert-parallel: host routes token(+slot) pairs to their expert, pads each
expert's batch to a common capacity C, pre-tiles the weights into the exact
contiguous chunks the kernel DMAs, and ships one expert per core.

PE floor at C=152 is 2688 matmuls x ~66ns = ~178us; weight ingest is sized
to fit under it.  Measured DMA rates: SWDGE cast-in-flight (int8 HBM read ->
fp16 SBUF write) sustains ~410 B/ns of SBUF writes; two HWDGE queues
aggregate ~410 B/ns.  Mix per 44.1M weight elements:

  - dma8 (SWDGE int8->fp16): bulk of w1 + tail of w3 + mid w2 chunks
  - eng  (int8 DMA + DVE/ACT/POOL cast): w3 body (casts run during phase A
    slack) + late w2 chunks (casts run during phase B when engines are free)
  - f16  (host-cast fp16 DMA): first groups of w1/w3 (fast pipeline prime)
    and the first 4 w2 chunks (instant phase-B start)

HBM reads ~48MB (135us), SBUF DMA writes ~77MB (~180us), casts ~12.3M
elems - all under / at the PE floor.  s1 is folded into the Silu via the
ACT engine's fused scale; t3 scaling uses DVE tensor_scalar.

Phase A per pair of i-tiles (both accumulated in ONE psum bank as [P,2,C]):
a = Silu(p1*s1) (ACT fused), t3 = p3*s3 (DVE), h = t3*a (DVE/POOL).
Phase B: y^T = (w2 @ h) * s2 with 4 psum banks, scale on DVE/ACT, DMA out.
"""

import os

import numpy as np

os.environ.setdefault("JAX_COMPILATION_CACHE_DIR", "/tmp/jax_cache")

# Problem constants (hardcoded per the task contract).
E = 8
D = 2048
I = 7168
P = 128

KD = D // P              # 16 contraction tiles for GEMM1/3
KI = I // P              # 56 i tiles
MT = D // P              # 16 output m tiles
PBM = 4                  # phase B m-tiles in flight (PSUM banks)
PBW = PBM * P            # 512: phase B weight chunk width (m cols)
MH = MT // PBM           # 4 phase-B m-groups
PBI = 8                  # phase B i-tiles per chunk
NB = KI // PBI           # 7 chunks per m-group
NCH = MH * NB            # 28 w2 chunks

# phase A i-group sizes (first groups small: the g0 transfers are the
# startup critical path)
GWS = [128, 384] + [512] * 13
assert sum(GWS) == I
# per-group sources: 'f16' host fp16, 'eng' int8 + engine cast,
# 'dma8' int8 + SWDGE DMA-cast (reads 1B/elem, writes 2B/elem).
# eng groups spread out (and w1/w3 eng offset from each other): cast load
# must stay within the engines' LOCAL per-period capacity.
W1_SRC = ['f16', 'f16'] + ['dma8'] * 13
W3_SRC = ['f16', 'f16'] + ['eng', 'dma8'] * 6 + ['dma8']
assert len(W1_SRC) == len(W3_SRC) == len(GWS)
# w2 chunks in consumption order; mh0 (c0-6, consumed while interleaved
# into phase A) avoids 'eng' (engines are cast-saturated in phase A);
# B' chunks rotate f16/eng/dma8 so sync, scalar and SWDGE each carry a
# third of the phase-B' ingest
W2_SRC = ['f16'] * 4 + ['dma8'] * 3 + ['dma8', 'eng', 'dma8'] * 7
assert len(W2_SRC) == NCH

_CACHE = {}
_LAST_RESULTS = None  # for test harness introspection


def _build_nc(C):
    import contextlib

    import concourse.bacc as bacc
    import concourse.tile as tile
    from concourse import mybir

    f16 = mybir.dt.float16
    f32 = mybir.dt.float32
    i8 = mybir.dt.int8

    assert C <= 256
    JSZ = 2

    nc = bacc.Bacc("TRN2", target_bir_lowering=False, debug=False, num_devices=E)

    xt = nc.dram_tensor("xt", [P, KD * C], f16, kind="ExternalInput").ap()
    # group-major partition-major weights; per group g the block is
    # [P, KD*gw] with value[p, k*gw+f] = w[gstart+f, k*P+p].  fp16 groups
    # and int8 groups (eng+dma8, in group order) pack separately.
    n1f = KD * sum(gw for gw, s in zip(GWS, W1_SRC) if s == 'f16')
    n1q = KD * sum(gw for gw, s in zip(GWS, W1_SRC) if s != 'f16')
    w1tf = nc.dram_tensor("w1tf", [P, max(n1f, 1)], f16, kind="ExternalInput").ap()
    w1tq = nc.dram_tensor("w1tq", [P, max(n1q, 1)], i8, kind="ExternalInput").ap()
    n3f = KD * sum(gw for gw, s in zip(GWS, W3_SRC) if s == 'f16')
    n3q = KD * sum(gw for gw, s in zip(GWS, W3_SRC) if s != 'f16')
    w3tf = nc.dram_tensor("w3tf", [P, max(n3f, 1)], f16, kind="ExternalInput").ap()
    w3tq = nc.dram_tensor("w3tq", [P, max(n3q, 1)], i8, kind="ExternalInput").ap()
    # phase B chunks: [P, PBI*PBW] per (mh, nb): value[p, i_l*PBW+f] =
    # w2[mh*PBW+f, (nb*PBI+i_l)*P+p]; fp16 / int8 chunks packed separately.
    n2f = PBI * PBW * sum(s == 'f16' for s in W2_SRC)
    n2q = PBI * PBW * sum(s != 'f16' for s in W2_SRC)
    w2tf = nc.dram_tensor("w2tf", [P, max(n2f, 1)], f16, kind="ExternalInput").ap()
    w2tq = nc.dram_tensor("w2tq", [P, max(n2q, 1)], i8, kind="ExternalInput").ap()
    s1 = nc.dram_tensor("s1", [P, KI], f32, kind="ExternalInput").ap()
    s3 = nc.dram_tensor("s3", [P, KI], f32, kind="ExternalInput").ap()
    s2 = nc.dram_tensor("s2", [P, MT], f32, kind="ExternalInput").ap()
    yt = nc.dram_tensor("yt", [D, C], f32, kind="ExternalOutput").ap()

    with tile.TileContext(nc) as tc:
        # greedy engine balancer: accumulated busy-ns per engine, costs from
        # measured rates (el/ns): cast act 44 / dve 51 / pool 34; dve fp16
        # mul ~90.  Fixed per-op overhead ~350-500ns (sem waits + dispatch).
        acc = {"act": 0.0, "dve": 0.0, "pool": 0.0}

        def pick(cost_ns, engines):
            best = min(engines, key=lambda e: acc[e] + cost_ns[e])
            acc[best] += cost_ns[best]
            return best

        def cast(out, in_, engines=("act", "dve", "pool")):
            elems = 128 * out.free_size()
            cost = {"act": elems / 44 + 400, "dve": elems / 51 + 350,
                    "pool": elems / 34 + 450}
            eng = pick(cost, engines)
            if eng == "act":
                nc.scalar.copy(out, in_)
            elif eng == "dve":
                nc.vector.tensor_copy(out, in_)
            else:
                nc.gpsimd.tensor_copy(out, in_)

        def cast_sliced(dstview, srcview, kd, gw, nsl):
            # split flat [P, kd*gw] into nsl k-slices balanced across engines
            bounds = [round(kd * t / nsl) for t in range(nsl + 1)]
            for a, b in zip(bounds, bounds[1:]):
                if b > a:
                    cast(dstview[:, a * gw:b * gw], srcview[:, a * gw:b * gw])

        def mul(out, a, b, engines=("dve", "pool")):
            elems = 128 * out.free_size()
            cost = {"dve": elems / 90 + 350, "pool": elems / 45 + 450}
            eng = pick(cost, engines)
            if eng == "dve":
                nc.vector.tensor_mul(out, a, b)
            else:
                nc.gpsimd.tensor_mul(out, a, b)

        with contextlib.ExitStack() as ctx:
            constp = ctx.enter_context(tc.tile_pool(name="const", bufs=1))
            w1p = ctx.enter_context(tc.tile_pool(name="w1p", bufs=3))
            w3sp = ctx.enter_context(tc.tile_pool(name="w3s", bufs=2))
            w3fp = ctx.enter_context(tc.tile_pool(name="w3f", bufs=3))
            hp = ctx.enter_context(tc.tile_pool(name="h", bufs=1))
            ep = ctx.enter_context(tc.tile_pool(name="eltw", bufs=2))
            w2sp = ctx.enter_context(tc.tile_pool(name="w2s", bufs=3))
            w2fp = ctx.enter_context(tc.tile_pool(name="w2f", bufs=4))
            outp = ctx.enter_context(tc.tile_pool(name="outp", bufs=4))

            # constants: x^T fp16 first on sync; scales on scalar queue
            xts = constp.tile([P, KD * C], f16)
            nc.sync.dma_start(xts, xt)
            s1s = constp.tile([P, KI], f32)
            s3s = constp.tile([P, KI], f32)
            s2s = constp.tile([P, MT], f32)

            h_index = {}   # i-tile index -> (tile, j)
            GWMAX = max(GWS)

            # ---- w2 chunk machinery.  'f16' chunks 0-3 DMA during phase A
            # (instant phase-B start); 'eng' chunks stage their int8 during
            # phase A (cheap SBUF) and cast during phase B when the eltwise
            # engines are otherwise free; 'dma8' chunks stream during phase B
            # via SWDGE.  w2f ring tiles are allocated strictly in
            # consumption order so ring reuse matches chunk drain order.
            f2off = [0]
            # int8 chunk offsets must follow chunk order (dma8+eng packed
            # together), so precompute them.
            _q2offs = {}
            _o = 0
            for _c, _s in enumerate(W2_SRC):
                if _s != 'f16':
                    _q2offs[_c] = _o
                    _o += PBI * PBW
            w2f_tiles = {}
            w2s_tiles = {}
            w2_casted = set()

            def emit_w2_dma(c):
                if c >= NCH or c in w2f_tiles:
                    return
                src = W2_SRC[c]
                w2f = w2fp.tile([P, PBI * PBW], f16, tag="w2f")
                if src == 'f16':
                    nc.sync.dma_start(
                        w2f, w2tf[:, f2off[0]:f2off[0] + PBI * PBW])
                    f2off[0] += PBI * PBW
                elif src == 'dma8':
                    nc.gpsimd.dma_start(
                        w2f, w2tq[:, _q2offs[c]:_q2offs[c] + PBI * PBW])
                    acc["pool"] += 900
                else:  # eng: int8 on scalar HWDGE (idle during phase B)
                    w2s8 = w2sp.tile([P, PBI * PBW], i8, tag="w2s8")
                    nc.scalar.dma_start(
                        w2s8, w2tq[:, _q2offs[c]:_q2offs[c] + PBI * PBW])
                    w2s_tiles[c] = w2s8
                w2f_tiles[c] = w2f

            def emit_w2_cast(c):
                if c >= NCH or c in w2_casted or c not in w2f_tiles:
                    return
                w2_casted.add(c)
                if c in w2s_tiles:
                    cast_sliced(w2f_tiles[c], w2s_tiles[c], PBI, PBW, 3)

            # ---------------- Phase A: h = silu(x@w1^T * s1) * (x@w3^T * s3)
            # software-pipelined emission: DMA group g+2, cast group g+1,
            # then compute group g, so casts never wait behind chain ops.
            f1off = [0]
            q1off = [0]
            f3off = [0]
            q3off = [0]
            w1f_t = {}
            w1s_t = {}
            w3f_t = {}
            w3s_t = {}
            w13_casted = set()

            def emit_one_dma(g, srcs, fpool, spool, ftab, stab, dramf, dramq,
                             foff, qoff, ftag, stag, hw_q):
                gw = GWS[g]
                wf = fpool.tile([P, KD * GWMAX], f16, tag=ftag)
                if srcs[g] == 'f16':
                    hw_q.dma_start(
                        wf[:, :KD * gw],
                        dramf[:, foff[0]:foff[0] + gw * KD])
                    foff[0] += gw * KD
                    if hw_q is nc.gpsimd:
                        acc["pool"] += 900
                elif srcs[g] == 'dma8':
                    nc.gpsimd.dma_start(
                        wf[:, :KD * gw],
                        dramq[:, qoff[0]:qoff[0] + gw * KD])
                    qoff[0] += gw * KD
                    acc["pool"] += 900
                else:  # eng
                    ws8 = spool.tile([P, KD * GWMAX], i8, tag=stag)
                    hw_q.dma_start(
                        ws8[:, :KD * gw],
                        dramq[:, qoff[0]:qoff[0] + gw * KD])
                    qoff[0] += gw * KD
                    stab[g] = ws8
                ftab[g] = wf

            # startup: first-group transfers are the PE critical path, so
            # give each queue one of them as its FIRST item (SDMA round-
            # robins across queues; a queue's share is ~1/3 of fabric).
            W1_Q = {0: nc.scalar, 1: nc.sync}
            W3_Q = {0: nc.gpsimd, 1: nc.scalar}

            def emit_w13_dma(g):
                if g >= len(GWS) or g in w1f_t:
                    return
                emit_one_dma(g, W1_SRC, w1p, w3sp, w1f_t, w1s_t,
                             w1tf, w1tq, f1off, q1off, "w1f", "w1s8",
                             W1_Q.get(g, nc.sync))
                emit_one_dma(g, W3_SRC, w3fp, w3sp, w3f_t, w3s_t,
                             w3tf, w3tq, f3off, q3off, "w3f", "w3s8",
                             W3_Q.get(g, nc.scalar))

            def emit_w13_cast(g):
                if g >= len(GWS) or g in w13_casted:
                    return
                w13_casted.add(g)
                gw = GWS[g]
                if g in w3s_t:
                    cast_sliced(w3f_t[g], w3s_t[g], KD, gw, 6)
                if g in w1s_t:
                    cast_sliced(w1f_t[g], w1s_t[g], KD, gw, 6)

            # w2 emission during phase A: mh0's chunks stream in as the
            # interleaved mh0 matmuls consume them.
            w2f16_sched = {1: [0], 2: [1], 3: [2], 4: [3], 6: [4],
                           8: [5], 10: [6]}

            # interleaved-mh0 state: ops (nb, i_l, ml) in i-major order
            m0ops = [(nb, i_l, ml) for nb in range(NB)
                     for i_l in range(PBI) for ml in range(PBM)]
            m0cur = [0]

            with tc.tile_pool(name="psA", bufs=2, space="PSUM") as psA, \
                 tc.tile_pool(name="psM0", bufs=1, space="PSUM") as psM0:
                pm0 = [psM0.tile([P, C], f32, tag=f"pm{ml}",
                                 name=f"pm0_{ml}") for ml in range(PBM)]

                def emit_m0(p, budget):
                    # emit up to `budget` mh0 matmuls whose h (i-tile q) was
                    # produced >= 2 pairs ago and whose w2 chunk is emitted
                    n = 0
                    while m0cur[0] < len(m0ops) and n < budget:
                        nb, i_l, ml = m0ops[m0cur[0]]
                        q = nb * PBI + i_l
                        if p is not None and q > 2 * p - 7:
                            break
                        if nb not in w2f_tiles:
                            break
                        ht, j = h_index[q]
                        nc.tensor.matmul(
                            pm0[ml],
                            w2f_tiles[nb][:, i_l * PBW + ml * P:
                                          i_l * PBW + (ml + 1) * P],
                            ht[:, j, :],
                            start=(q == 0), stop=(q == KI - 1))
                        m0cur[0] += 1
                        n += 1

                emit_w13_dma(0)
                emit_w13_dma(1)
                emit_w13_dma(2)
                nc.sync.dma_start(s1s, s1)
                nc.sync.dma_start(s3s, s3)
                nc.sync.dma_start(s2s, s2)
                emit_w13_cast(0)
                emit_w13_cast(1)
                goff = 0
                for g, gw in enumerate(GWS):
                    il0 = goff // P
                    nil = gw // P
                    emit_w13_dma(g + 2)   # usually a no-op (see loop tail)
                    emit_w13_cast(g + 1)
                    for c in w2f16_sched.get(g, []):
                        emit_w2_dma(c)
                    w1f = w1f_t[g]
                    w3f = w3f_t[g]

                    il = 0
                    while il < nil:
                        jn = min(2, nil - il)
                        i = il0 + il
                        p1 = psA.tile([P, JSZ, C], f32, tag="p1")
                        p3 = psA.tile([P, JSZ, C], f32, tag="p3")
                        for j in range(jn):
                            lo = (il + j) * P
                            for k in range(KD):
                                nc.tensor.matmul(
                                    p1[:, j, :],
                                    w1f[:, k * gw + lo:k * gw + lo + P],
                                    xts[:, k * C:(k + 1) * C],
                                    start=(k == 0), stop=(k == KD - 1))
                        for j in range(jn):
                            lo = (il + j) * P
                            for k in range(KD):
                                nc.tensor.matmul(
                                    p3[:, j, :],
                                    w3f[:, k * gw + lo:k * gw + lo + P],
                                    xts[:, k * C:(k + 1) * C],
                                    start=(k == 0), stop=(k == KD - 1))
                        # a = silu(p1 * s1): fused scale on ACT, per j
                        a = ep.tile([P, JSZ, C], f16, tag="a")
                        for j in range(jn):
                            nc.scalar.activation(
                                a[:, j, :], p1[:, j, :],
                                mybir.ActivationFunctionType.Silu,
                                scale=s1s[:, i + j:i + j + 1])
                            acc["act"] += 128 * C / 44 + 400
                        # t3 = p3 * s3 on DVE (batched broadcast mul)
                        t3 = ep.tile([P, JSZ, C], f16, tag="t3")
                        nc.vector.tensor_mul(
                            t3[:, :jn, :], p3[:, :jn, :],
                            s3s[:, i:i + jn]
                            .rearrange("p (k o) -> p k o", o=1)
                            .broadcast_to([P, jn, C]))
                        acc["dve"] += 128 * jn * C / 90 + 350
                        htile = hp.tile([P, JSZ, C], f16, tag=f"h{i}")
                        mul(htile[:, :jn, :], t3[:, :jn, :], a[:, :jn, :])
                        for j in range(jn):
                            h_index[i + j] = (htile, j)
                        il += jn
                        emit_m0(i // 2, 10)
                    # deepest prefetch AFTER group g's consumers are emitted,
                    # so the ring-slot WAR dependency is recorded correctly
                    emit_w13_dma(g + 3)
                    emit_w13_cast(g + 2)
                    goff += gw
                # flush the remaining mh0 matmuls (last pairs' h)
                emit_m0(None, len(m0ops))
                for ml in range(PBM):
                    m = ml
                    o = outp.tile([P, C], f32, tag="o")
                    if ml % 2 == 0:
                        nc.vector.tensor_scalar_mul(o, pm0[ml], s2s[:, m:m + 1])
                    else:
                        nc.scalar.mul(o, pm0[ml], s2s[:, m:m + 1])
                    q = nc.sync if ml % 2 == 0 else nc.scalar
                    q.dma_start(yt[m * P:(m + 1) * P, :], o)

            # ---------------- Phase B: y^T = (w2 @ h) * s2 for mh 1..3
            with tc.tile_pool(name="psB", bufs=2, space="PSUM") as psB:
                for mh in range(1, MH):
                    pbs = [psB.tile([P, C], f32, tag=f"pb{ml}",
                                    name=f"pb{mh}_{ml}")
                           for ml in range(PBM)]
                    for nb in range(NB):
                        c = mh * NB + nb
                        emit_w2_dma(c)       # no-op unless pipeline fell behind
                        emit_w2_cast(c)
                        emit_w2_dma(c + 3)
                        emit_w2_cast(c + 1)
                        emit_w2_cast(c + 2)
                        emit_w2_cast(c + 3)
                        w2f = w2f_tiles[c]
                        for i_l in range(PBI):
                            i = nb * PBI + i_l
                            ht, j = h_index[i]
                            for ml in range(PBM):
                                nc.tensor.matmul(
                                    pbs[ml],
                                    w2f[:, i_l * PBW + ml * P:
                                        i_l * PBW + (ml + 1) * P],
                                    ht[:, j, :],
                                    start=(i == 0), stop=(i == KI - 1))
                        # deepest prefetch after chunk c's matmuls (ring WAR)
                        emit_w2_dma(c + 4)
                        emit_w2_cast(c + 4)
                    for ml in range(PBM):
                        m = mh * PBM + ml
                        o = outp.tile([P, C], f32, tag="o")
                        if ml % 2 == 0:
                            nc.vector.tensor_scalar_mul(
                                o, pbs[ml], s2s[:, m:m + 1])
                        else:
                            nc.scalar.mul(o, pbs[ml], s2s[:, m:m + 1])
                        q = nc.sync if ml % 2 == 0 else nc.scalar
                        q.dma_start(yt[m * P:(m + 1) * P, :], o)

    nc.compile()
    return nc


def _tile_w13(w, srcs):
    """[I, D] -> (fp16 blocks, int8 blocks) packed per GWS/srcs."""
    fparts, qparts = [], []
    goff = 0
    for gw, s in zip(GWS, srcs):
        blk = w[goff:goff + gw, :]                      # [gw, D] int8
        t = blk.reshape(gw, KD, P).transpose(2, 1, 0).reshape(P, KD * gw)
        if s == 'f16':
            fparts.append(t.astype(np.float16))
        else:
            qparts.append(t)
        goff += gw
    fcat = (np.ascontiguousarray(np.concatenate(fparts, axis=1))
            if fparts else np.zeros((P, 1), np.float16))
    qcat = (np.ascontiguousarray(np.concatenate(qparts, axis=1))
            if qparts else np.zeros((P, 1), np.int8))
    return fcat, qcat


def _tile_w2(w):
    """[D, I] int8 -> (fp16 chunks, int8 chunks) packed per W2_SRC."""
    t = w.reshape(MH, PBW, KI, P).transpose(0, 3, 2, 1)  # [MH, P, KI, PBW]
    fparts, qparts = [], []
    for mh in range(MH):
        for nb in range(NB):
            c = mh * NB + nb
            blk = t[mh, :, nb * PBI:(nb + 1) * PBI, :].reshape(P, PBI * PBW)
            if W2_SRC[c] == 'f16':
                fparts.append(blk.astype(np.float16))
            else:
                qparts.append(blk)
    fcat = (np.ascontiguousarray(np.concatenate(fparts, axis=1))
            if fparts else np.zeros((P, 1), np.float16))
    qcat = (np.ascontiguousarray(np.concatenate(qparts, axis=1))
            if qparts else np.zeros((P, 1), np.int8))
    return fcat, qcat


def _route(expert_indices):
    idx = np.asarray(expert_indices).astype(np.int64)
    toks, slots = [], []
    for e in range(E):
        t, a = np.nonzero(idx == e)
        toks.append(t)
        slots.append(a)
    return toks, slots


def _prepare(inputs):
    x = np.asarray(inputs["x"], dtype=np.float32)          # [T, D]
    expert_indices = np.asarray(inputs["expert_indices"])  # [T, A]
    w1 = np.asarray(inputs["w1"])                          # [E, I, D] int8
    w2 = np.asarray(inputs["w2"])                          # [E, D, I] int8
    w3 = np.asarray(inputs["w3"])                          # [E, I, D] int8
    scales1 = np.asarray(inputs["scales1"], dtype=np.float32)
    scales2 = np.asarray(inputs["scales2"], dtype=np.float32)
    scales3 = np.asarray(inputs["scales3"], dtype=np.float32)

    T, A = expert_indices.shape
    toks, slots = _route(expert_indices)
    counts = [len(t) for t in toks]
    C = max(max(counts), 8)
    C = (C + 1) // 2 * 2

    if C not in _CACHE:
        _CACHE[C] = _build_nc(C)
    nc = _CACHE[C]

    in_maps = []
    for e in range(E):
        n_e = counts[e]
        xtc = np.zeros((P, KD, C), dtype=np.float16)
        if n_e:
            xe = x[toks[e]].astype(np.float16)             # [n_e, D]
            xtc[:, :, :n_e] = xe.T.reshape(KD, P, n_e).transpose(1, 0, 2)
        w1f, w1q = _tile_w13(w1[e], W1_SRC)
        w3f, w3q = _tile_w13(w3[e], W3_SRC)
        w2f, w2q = _tile_w2(w2[e])
        in_maps.append(
            dict(
                xt=np.ascontiguousarray(xtc.reshape(P, KD * C)),
                w1tf=w1f,
                w1tq=w1q,
                w3tf=w3f,
                w3tq=w3q,
                w2tf=w2f,
                w2tq=w2q,
                s1=np.ascontiguousarray(scales1[e].reshape(KI, P).T),
                s3=np.ascontiguousarray(scales3[e].reshape(KI, P).T),
                s2=np.ascontiguousarray(scales2[e].reshape(MT, P).T),
            )
        )
    return nc, in_maps, (T, A, toks, slots, counts)


def kernel(**inputs):
    global _LAST_RESULTS
    from concourse.bass_utils import run_bass_kernel_spmd

    nc, in_maps, (T, A, toks, slots, counts) = _prepare(inputs)
    res = run_bass_kernel_spmd(nc, in_maps, core_ids=list(range(E)))
    _LAST_RESULTS = res

    out = np.zeros((T, A, D), dtype=np.float32)
    for e in range(E):
        n_e = counts[e]
        if n_e:
            ye = res.results[e]["yt"][:, :n_e].T  # [n_e, D]
            out[toks[e], slots[e], :] = ye
    return out


# revision 18
# speedup vs baseline: 1.0674x; 1.0674x over previous
"""MoE ConditionalFeedForward (int8 SwiGLU experts) on 8 trn2 NeuronCores.

Expert-parallel: host routes token(+slot) pairs to their expert, pads each
expert's batch to a common capacity C, pre-tiles the weights into the exact
contiguous chunks the kernel DMAs, and ships one expert per core.

PE floor at C=152 is 2688 matmuls x ~66ns = ~178us; weight ingest is sized
to fit under it.  Measured DMA rates: SWDGE cast-in-flight (int8 HBM read ->
fp16 SBUF write) sustains ~410 B/ns of SBUF writes; two HWDGE queues
aggregate ~410 B/ns.  Mix per 44.1M weight elements:

  - dma8 (SWDGE int8->fp16): bulk of w1 + tail of w3 + mid w2 chunks
  - eng  (int8 DMA + DVE/ACT/POOL cast): w3 body (casts run during phase A
    slack) + late w2 chunks (casts run during phase B when engines are free)
  - f16  (host-cast fp16 DMA): first groups of w1/w3 (fast pipeline prime)
    and the first 4 w2 chunks (instant phase-B start)

HBM reads ~48MB (135us), SBUF DMA writes ~77MB (~180us), casts ~12.3M
elems - all under / at the PE floor.  s1 is folded into the Silu via the
ACT engine's fused scale; t3 scaling uses DVE tensor_scalar.

Phase A per pair of i-tiles (both accumulated in ONE psum bank as [P,2,C]):
a = Silu(p1*s1) (ACT fused), t3 = p3*s3 (DVE), h = t3*a (DVE/POOL).
Phase B: y^T = (w2 @ h) * s2 with 4 psum banks, scale on DVE/ACT, DMA out.
"""

import os

import numpy as np

os.environ.setdefault("JAX_COMPILATION_CACHE_DIR", "/tmp/jax_cache")

# Problem constants (hardcoded per the task contract).
E = 8
D = 2048
I = 7168
P = 128

KD = D // P              # 16 contraction tiles for GEMM1/3
KI = I // P              # 56 i tiles
MT = D // P              # 16 output m tiles
PBM = 4                  # phase B m-tiles in flight (PSUM banks)
PBW = PBM * P            # 512: phase B weight chunk width (m cols)
MH = MT // PBM           # 4 phase-B m-groups
PBI = 8                  # phase B i-tiles per chunk
NB = KI // PBI           # 7 chunks per m-group
NCH = MH * NB            # 28 w2 chunks

# phase A i-group sizes (uniform: full-width tiles, clean DMA runs)
GWS = [512] * 14
assert sum(GWS) == I
# per-group sources: 'f16' host fp16, 'eng' int8 + engine cast,
# 'dma8' int8 + SWDGE DMA-cast (reads 1B/elem, writes 2B/elem)
W1_SRC = ['f16', 'f16'] + ['dma8'] * 12
# eng groups interleaved with dma8: cast load must stay within the
# engines' LOCAL per-period capacity, not just the phase-A total
W3_SRC = ['f16', 'f16'] + ['eng', 'dma8'] * 6
assert len(W1_SRC) == len(W3_SRC) == len(GWS)
# w2 chunks in consumption order; mh0 (c0-6, consumed while interleaved
# into phase A) avoids 'eng' (engines are cast-saturated in phase A)
W2_SRC = ['f16'] * 4 + ['dma8'] * 3 + ['dma8', 'eng', 'dma8'] * 7
assert len(W2_SRC) == NCH

_CACHE = {}
_LAST_RESULTS = None  # for test harness introspection


def _build_nc(C):
    import contextlib

    import concourse.bacc as bacc
    import concourse.tile as tile
    from concourse import mybir

    f16 = mybir.dt.float16
    f32 = mybir.dt.float32
    i8 = mybir.dt.int8

    assert C <= 256
    JSZ = 2

    nc = bacc.Bacc("TRN2", target_bir_lowering=False, debug=False, num_devices=E)

    xt = nc.dram_tensor("xt", [P, KD * C], f16, kind="ExternalInput").ap()
    # group-major partition-major weights; per group g the block is
    # [P, KD*gw] with value[p, k*gw+f] = w[gstart+f, k*P+p].  fp16 groups
    # and int8 groups (eng+dma8, in group order) pack separately.
    n1f = KD * sum(gw for gw, s in zip(GWS, W1_SRC) if s == 'f16')
    n1q = KD * sum(gw for gw, s in zip(GWS, W1_SRC) if s != 'f16')
    w1tf = nc.dram_tensor("w1tf", [P, max(n1f, 1)], f16, kind="ExternalInput").ap()
    w1tq = nc.dram_tensor("w1tq", [P, max(n1q, 1)], i8, kind="ExternalInput").ap()
    n3f = KD * sum(gw for gw, s in zip(GWS, W3_SRC) if s == 'f16')
    n3q = KD * sum(gw for gw, s in zip(GWS, W3_SRC) if s != 'f16')
    w3tf = nc.dram_tensor("w3tf", [P, max(n3f, 1)], f16, kind="ExternalInput").ap()
    w3tq = nc.dram_tensor("w3tq", [P, max(n3q, 1)], i8, kind="ExternalInput").ap()
    # phase B chunks: [P, PBI*PBW] per (mh, nb): value[p, i_l*PBW+f] =
    # w2[mh*PBW+f, (nb*PBI+i_l)*P+p]; fp16 / int8 chunks packed separately.
    n2f = PBI * PBW * sum(s == 'f16' for s in W2_SRC)
    n2q = PBI * PBW * sum(s != 'f16' for s in W2_SRC)
    w2tf = nc.dram_tensor("w2tf", [P, max(n2f, 1)], f16, kind="ExternalInput").ap()
    w2tq = nc.dram_tensor("w2tq", [P, max(n2q, 1)], i8, kind="ExternalInput").ap()
    s1 = nc.dram_tensor("s1", [P, KI], f32, kind="ExternalInput").ap()
    s3 = nc.dram_tensor("s3", [P, KI], f32, kind="ExternalInput").ap()
    s2 = nc.dram_tensor("s2", [P, MT], f32, kind="ExternalInput").ap()
    yt = nc.dram_tensor("yt", [D, C], f32, kind="ExternalOutput").ap()

    with tile.TileContext(nc) as tc:
        # greedy engine balancer: accumulated busy-ns per engine, costs from
        # measured rates (el/ns): cast act 44 / dve 51 / pool 34; dve fp16
        # mul ~90.  Fixed per-op overhead ~350-500ns (sem waits + dispatch).
        acc = {"act": 0.0, "dve": 0.0, "pool": 0.0}

        def pick(cost_ns, engines):
            best = min(engines, key=lambda e: acc[e] + cost_ns[e])
            acc[best] += cost_ns[best]
            return best

        def cast(out, in_, engines=("act", "dve", "pool")):
            elems = 128 * out.free_size()
            cost = {"act": elems / 44 + 400, "dve": elems / 51 + 350,
                    "pool": elems / 34 + 450}
            eng = pick(cost, engines)
            if eng == "act":
                nc.scalar.copy(out, in_)
            elif eng == "dve":
                nc.vector.tensor_copy(out, in_)
            else:
                nc.gpsimd.tensor_copy(out, in_)

        def cast_sliced(dstview, srcview, kd, gw, nsl):
            # split flat [P, kd*gw] into nsl k-slices balanced across engines
            bounds = [round(kd * t / nsl) for t in range(nsl + 1)]
            for a, b in zip(bounds, bounds[1:]):
                if b > a:
                    cast(dstview[:, a * gw:b * gw], srcview[:, a * gw:b * gw])

        def mul(out, a, b, engines=("dve", "pool")):
            elems = 128 * out.free_size()
            cost = {"dve": elems / 90 + 350, "pool": elems / 45 + 450}
            eng = pick(cost, engines)
            if eng == "dve":
                nc.vector.tensor_mul(out, a, b)
            else:
                nc.gpsimd.tensor_mul(out, a, b)

        with contextlib.ExitStack() as ctx:
            constp = ctx.enter_context(tc.tile_pool(name="const", bufs=1))
            w1p = ctx.enter_context(tc.tile_pool(name="w1p", bufs=3))
            w3sp = ctx.enter_context(tc.tile_pool(name="w3s", bufs=2))
            w3fp = ctx.enter_context(tc.tile_pool(name="w3f", bufs=3))
            hp = ctx.enter_context(tc.tile_pool(name="h", bufs=1))
            ep = ctx.enter_context(tc.tile_pool(name="eltw", bufs=2))
            w2sp = ctx.enter_context(tc.tile_pool(name="w2s", bufs=4))
            w2fp = ctx.enter_context(tc.tile_pool(name="w2f", bufs=5))
            outp = ctx.enter_context(tc.tile_pool(name="outp", bufs=4))

            # constants: x^T fp16 first on sync; scales on scalar queue
            xts = constp.tile([P, KD * C], f16)
            nc.sync.dma_start(xts, xt)
            s1s = constp.tile([P, KI], f32)
            s3s = constp.tile([P, KI], f32)
            s2s = constp.tile([P, MT], f32)

            h_index = {}   # i-tile index -> (tile, j)
            GWMAX = max(GWS)

            # ---- w2 chunk machinery.  'f16' chunks 0-3 DMA during phase A
            # (instant phase-B start); 'eng' chunks stage their int8 during
            # phase A (cheap SBUF) and cast during phase B when the eltwise
            # engines are otherwise free; 'dma8' chunks stream during phase B
            # via SWDGE.  w2f ring tiles are allocated strictly in
            # consumption order so ring reuse matches chunk drain order.
            f2off = [0]
            # int8 chunk offsets must follow chunk order (dma8+eng packed
            # together), so precompute them.
            _q2offs = {}
            _o = 0
            for _c, _s in enumerate(W2_SRC):
                if _s != 'f16':
                    _q2offs[_c] = _o
                    _o += PBI * PBW
            w2f_tiles = {}
            w2s_tiles = {}
            w2_casted = set()

            def emit_w2_dma(c):
                if c >= NCH or c in w2f_tiles:
                    return
                src = W2_SRC[c]
                w2f = w2fp.tile([P, PBI * PBW], f16, tag="w2f")
                if src == 'f16':
                    nc.sync.dma_start(
                        w2f, w2tf[:, f2off[0]:f2off[0] + PBI * PBW])
                    f2off[0] += PBI * PBW
                elif src == 'dma8':
                    nc.gpsimd.dma_start(
                        w2f, w2tq[:, _q2offs[c]:_q2offs[c] + PBI * PBW])
                    acc["pool"] += 900
                else:  # eng: int8 on scalar HWDGE (idle during phase B)
                    w2s8 = w2sp.tile([P, PBI * PBW], i8, tag="w2s8")
                    nc.scalar.dma_start(
                        w2s8, w2tq[:, _q2offs[c]:_q2offs[c] + PBI * PBW])
                    w2s_tiles[c] = w2s8
                w2f_tiles[c] = w2f

            def emit_w2_cast(c):
                if c >= NCH or c in w2_casted or c not in w2f_tiles:
                    return
                w2_casted.add(c)
                if c in w2s_tiles:
                    cast_sliced(w2f_tiles[c], w2s_tiles[c], PBI, PBW, 3)

            # ---------------- Phase A: h = silu(x@w1^T * s1) * (x@w3^T * s3)
            # software-pipelined emission: DMA group g+2, cast group g+1,
            # then compute group g, so casts never wait behind chain ops.
            f1off = [0]
            q1off = [0]
            f3off = [0]
            q3off = [0]
            w1f_t = {}
            w1s_t = {}
            w3f_t = {}
            w3s_t = {}
            w13_casted = set()

            def emit_one_dma(g, srcs, fpool, spool, ftab, stab, dramf, dramq,
                             foff, qoff, ftag, stag, hw_q):
                gw = GWS[g]
                wf = fpool.tile([P, KD * GWMAX], f16, tag=ftag)
                if srcs[g] == 'f16':
                    hw_q.dma_start(
                        wf[:, :KD * gw],
                        dramf[:, foff[0]:foff[0] + gw * KD])
                    foff[0] += gw * KD
                    if hw_q is nc.gpsimd:
                        acc["pool"] += 900
                elif srcs[g] == 'dma8':
                    nc.gpsimd.dma_start(
                        wf[:, :KD * gw],
                        dramq[:, qoff[0]:qoff[0] + gw * KD])
                    qoff[0] += gw * KD
                    acc["pool"] += 900
                else:  # eng
                    ws8 = spool.tile([P, KD * GWMAX], i8, tag=stag)
                    hw_q.dma_start(
                        ws8[:, :KD * gw],
                        dramq[:, qoff[0]:qoff[0] + gw * KD])
                    qoff[0] += gw * KD
                    stab[g] = ws8
                ftab[g] = wf

            # startup: first-group transfers are the PE critical path, so
            # give each queue one of them as its FIRST item (SDMA round-
            # robins across queues; a queue's share is ~1/3 of fabric).
            W1_Q = {0: nc.scalar, 1: nc.sync}
            W3_Q = {0: nc.gpsimd, 1: nc.scalar}

            def emit_w13_dma(g):
                if g >= len(GWS) or g in w1f_t:
                    return
                emit_one_dma(g, W1_SRC, w1p, None, w1f_t, w1s_t,
                             w1tf, w1tq, f1off, q1off, "w1f", "w1s8",
                             W1_Q.get(g, nc.sync))
                emit_one_dma(g, W3_SRC, w3fp, w3sp, w3f_t, w3s_t,
                             w3tf, w3tq, f3off, q3off, "w3f", "w3s8",
                             W3_Q.get(g, nc.scalar))

            def emit_w13_cast(g):
                if g >= len(GWS) or g in w13_casted:
                    return
                w13_casted.add(g)
                gw = GWS[g]
                if g in w3s_t:
                    cast_sliced(w3f_t[g], w3s_t[g], KD, gw, 6)
                if g in w1s_t:
                    cast_sliced(w1f_t[g], w1s_t[g], KD, gw, 6)

            # w2 emission during phase A: mh0's chunks stream in as the
            # interleaved mh0 matmuls consume them.
            w2f16_sched = {1: [0], 2: [1], 3: [2], 4: [3], 5: [4],
                           7: [5], 9: [6]}

            # interleaved-mh0 state: ops (nb, i_l, ml) in i-major order
            m0ops = [(nb, i_l, ml) for nb in range(NB)
                     for i_l in range(PBI) for ml in range(PBM)]
            m0cur = [0]

            with tc.tile_pool(name="psA", bufs=2, space="PSUM") as psA, \
                 tc.tile_pool(name="psM0", bufs=1, space="PSUM") as psM0:
                pm0 = [psM0.tile([P, C], f32, tag=f"pm{ml}",
                                 name=f"pm0_{ml}") for ml in range(PBM)]

                def emit_m0(p, budget):
                    # emit up to `budget` mh0 matmuls whose h (i-tile q) was
                    # produced >= 2 pairs ago and whose w2 chunk is emitted
                    n = 0
                    while m0cur[0] < len(m0ops) and n < budget:
                        nb, i_l, ml = m0ops[m0cur[0]]
                        q = nb * PBI + i_l
                        if p is not None and q > 2 * p - 3:
                            break
                        if nb not in w2f_tiles:
                            break
                        ht, j = h_index[q]
                        nc.tensor.matmul(
                            pm0[ml],
                            w2f_tiles[nb][:, i_l * PBW + ml * P:
                                          i_l * PBW + (ml + 1) * P],
                            ht[:, j, :],
                            start=(q == 0), stop=(q == KI - 1))
                        m0cur[0] += 1
                        n += 1

                emit_w13_dma(0)
                emit_w13_dma(1)
                emit_w13_dma(2)
                nc.sync.dma_start(s1s, s1)
                nc.sync.dma_start(s3s, s3)
                nc.sync.dma_start(s2s, s2)
                emit_w13_cast(0)
                emit_w13_cast(1)
                goff = 0
                for g, gw in enumerate(GWS):
                    il0 = goff // P
                    nil = gw // P
                    emit_w13_dma(g + 2)   # usually a no-op (see loop tail)
                    emit_w13_cast(g + 1)
                    for c in w2f16_sched.get(g, []):
                        emit_w2_dma(c)
                    w1f = w1f_t[g]
                    w3f = w3f_t[g]

                    il = 0
                    while il < nil:
                        jn = min(2, nil - il)
                        i = il0 + il
                        p1 = psA.tile([P, JSZ, C], f32, tag="p1")
                        p3 = psA.tile([P, JSZ, C], f32, tag="p3")
                        for j in range(jn):
                            lo = (il + j) * P
                            for k in range(KD):
                                nc.tensor.matmul(
                                    p1[:, j, :],
                                    w1f[:, k * gw + lo:k * gw + lo + P],
                                    xts[:, k * C:(k + 1) * C],
                                    start=(k == 0), stop=(k == KD - 1))
                        for j in range(jn):
                            lo = (il + j) * P
                            for k in range(KD):
                                nc.tensor.matmul(
                                    p3[:, j, :],
                                    w3f[:, k * gw + lo:k * gw + lo + P],
                                    xts[:, k * C:(k + 1) * C],
                                    start=(k == 0), stop=(k == KD - 1))
                        # a = silu(p1 * s1): fused scale on ACT, per j
                        a = ep.tile([P, JSZ, C], f16, tag="a")
                        for j in range(jn):
                            nc.scalar.activation(
                                a[:, j, :], p1[:, j, :],
                                mybir.ActivationFunctionType.Silu,
                                scale=s1s[:, i + j:i + j + 1])
                            acc["act"] += 128 * C / 44 + 400
                        # t3 = p3 * s3 on DVE (batched broadcast mul)
                        t3 = ep.tile([P, JSZ, C], f16, tag="t3")
                        nc.vector.tensor_mul(
                            t3[:, :jn, :], p3[:, :jn, :],
                            s3s[:, i:i + jn]
                            .rearrange("p (k o) -> p k o", o=1)
                            .broadcast_to([P, jn, C]))
                        acc["dve"] += 128 * jn * C / 90 + 350
                        htile = hp.tile([P, JSZ, C], f16, tag=f"h{i}")
                        mul(htile[:, :jn, :], t3[:, :jn, :], a[:, :jn, :])
                        for j in range(jn):
                            h_index[i + j] = (htile, j)
                        il += jn
                        emit_m0(i // 2, 10)
                    # deepest prefetch AFTER group g's consumers are emitted,
                    # so the ring-slot WAR dependency is recorded correctly
                    emit_w13_dma(g + 3)
                    emit_w13_cast(g + 2)
                    goff += gw
                # flush the remaining mh0 matmuls (last pairs' h)
                emit_m0(None, len(m0ops))
                for ml in range(PBM):
                    m = ml
                    o = outp.tile([P, C], f32, tag="o")
                    if ml % 2 == 0:
                        nc.vector.tensor_scalar_mul(o, pm0[ml], s2s[:, m:m + 1])
                    else:
                        nc.scalar.mul(o, pm0[ml], s2s[:, m:m + 1])
                    q = nc.sync if ml % 2 == 0 else nc.scalar
                    q.dma_start(yt[m * P:(m + 1) * P, :], o)

            # ---------------- Phase B: y^T = (w2 @ h) * s2 for mh 1..3
            with tc.tile_pool(name="psB", bufs=2, space="PSUM") as psB:
                for mh in range(1, MH):
                    pbs = [psB.tile([P, C], f32, tag=f"pb{ml}",
                                    name=f"pb{mh}_{ml}")
                           for ml in range(PBM)]
                    for nb in range(NB):
                        c = mh * NB + nb
                        emit_w2_dma(c)       # no-op unless pipeline fell behind
                        emit_w2_cast(c)
                        emit_w2_dma(c + 3)
                        emit_w2_dma(c + 4)
                        emit_w2_cast(c + 1)
                        emit_w2_cast(c + 2)
                        emit_w2_cast(c + 3)
                        emit_w2_cast(c + 4)
                        w2f = w2f_tiles[c]
                        for i_l in range(PBI):
                            i = nb * PBI + i_l
                            ht, j = h_index[i]
                            for ml in range(PBM):
                                nc.tensor.matmul(
                                    pbs[ml],
                                    w2f[:, i_l * PBW + ml * P:
                                        i_l * PBW + (ml + 1) * P],
                                    ht[:, j, :],
                                    start=(i == 0), stop=(i == KI - 1))
                        # deepest prefetch after chunk c's matmuls (ring WAR)
                        emit_w2_dma(c + 5)
                        emit_w2_cast(c + 5)
                    for ml in range(PBM):
                        m = mh * PBM + ml
                        o = outp.tile([P, C], f32, tag="o")
                        if ml % 2 == 0:
                            nc.vector.tensor_scalar_mul(
                                o, pbs[ml], s2s[:, m:m + 1])
                        else:
                            nc.scalar.mul(o, pbs[ml], s2s[:, m:m + 1])
                        q = nc.sync if ml % 2 == 0 else nc.scalar
                        q.dma_start(yt[m * P:(m + 1) * P, :], o)

    nc.compile()
    return nc


def _tile_w13(w, srcs):
    """[I, D] -> (fp16 blocks, int8 blocks) packed per GWS/srcs."""
    fparts, qparts = [], []
    goff = 0
    for gw, s in zip(GWS, srcs):
        blk = w[goff:goff + gw, :]                      # [gw, D] int8
        t = blk.reshape(gw, KD, P).transpose(2, 1, 0).reshape(P, KD * gw)
        if s == 'f16':
            fparts.append(t.astype(np.float16))
        else:
            qparts.append(t)
        goff += gw
    fcat = (np.ascontiguousarray(np.concatenate(fparts, axis=1))
            if fparts else np.zeros((P, 1), np.float16))
    qcat = (np.ascontiguousarray(np.concatenate(qparts, axis=1))
            if qparts else np.zeros((P, 1), np.int8))
    return fcat, qcat


def _tile_w2(w):
    """[D, I] int8 -> (fp16 chunks, int8 chunks) packed per W2_SRC."""
    t = w.reshape(MH, PBW, KI, P).transpose(0, 3, 2, 1)  # [MH, P, KI, PBW]
    fparts, qparts = [], []
    for mh in range(MH):
        for nb in range(NB):
            c = mh * NB + nb
            blk = t[mh, :, nb * PBI:(nb + 1) * PBI, :].reshape(P, PBI * PBW)
            if W2_SRC[c] == 'f16':
                fparts.append(blk.astype(np.float16))
            else:
                qparts.append(blk)
    fcat = (np.ascontiguousarray(np.concatenate(fparts, axis=1))
            if fparts else np.zeros((P, 1), np.float16))
    qcat = (np.ascontiguousarray(np.concatenate(qparts, axis=1))
            if qparts else np.zeros((P, 1), np.int8))
    return fcat, qcat


def _route(expert_indices):
    idx = np.asarray(expert_indices).astype(np.int64)
    toks, slots = [], []
    for e in range(E):
        t, a = np.nonzero(idx == e)
        toks.append(t)
        slots.append(a)
    return toks, slots


def _prepare(inputs):
    x = np.asarray(inputs["x"], dtype=np.float32)          # [T, D]
    expert_indices = np.asarray(inputs["expert_indices"])  # [T, A]
    w1 = np.asarray(inputs["w1"])                          # [E, I, D] int8
    w2 = np.asarray(inputs["w2"])                          # [E, D, I] int8
    w3 = np.asarray(inputs["w3"])                          # [E, I, D] int8
    scales1 = np.asarray(inputs["scales1"], dtype=np.float32)
    scales2 = np.asarray(inputs["scales2"], dtype=np.float32)
    scales3 = np.asarray(inputs["scales3"], dtype=np.float32)

    T, A = expert_indices.shape
    toks, slots = _route(expert_indices)
    counts = [len(t) for t in toks]
    C = max(max(counts), 8)
    C = (C + 1) // 2 * 2

    if C not in _CACHE:
        _CACHE[C] = _build_nc(C)
    nc = _CACHE[C]

    in_maps = []
    for e in range(E):
        n_e = counts[e]
        xtc = np.zeros((P, KD, C), dtype=np.float16)
        if n_e:
            xe = x[toks[e]].astype(np.float16)             # [n_e, D]
            xtc[:, :, :n_e] = xe.T.reshape(KD, P, n_e).transpose(1, 0, 2)
        w1f, w1q = _tile_w13(w1[e], W1_SRC)
        w3f, w3q = _tile_w13(w3[e], W3_SRC)
        w2f, w2q = _tile_w2(w2[e])
        in_maps.append(
            dict(
                xt=np.ascontiguousarray(xtc.reshape(P, KD * C)),
                w1tf=w1f,
                w1tq=w1q,
                w3tf=w3f,
                w3tq=w3q,
                w2tf=w2f,
                w2tq=w2q,
                s1=np.ascontiguousarray(scales1[e].reshape(KI, P).T),
                s3=np.ascontiguousarray(scales3[e].reshape(KI, P).T),
                s2=np.ascontiguousarray(scales2[e].reshape(MT, P).T),
            )
        )
    return nc, in_maps, (T, A, toks, slots, counts)


def kernel(**inputs):
    global _LAST_RESULTS
    from concourse.bass_utils import run_bass_kernel_spmd

    nc, in_maps, (T, A, toks, slots, counts) = _prepare(inputs)
    res = run_bass_kernel_spmd(nc, in_maps, core_ids=list(range(E)))
    _LAST_RESULTS = res

    out = np.zeros((T, A, D), dtype=np.float32)
    for e in range(E):
        n_e = counts[e]
        if n_e:
            ye = res.results[e]["yt"][:, :n_e].T  # [n_e, D]
            out[toks[e], slots[e], :] = ye
    return out
